# revision 18
# baseline (speedup 1.0000x reference)
"""Trainium2 Bass kernel for nn_PinyinGRUEmbeddings.

Math: x = emb_eff[tokens] ([B,T,8], emb row 0 zeroed), two stacked GRU
layers (torch gate order r,z,n), output = layer-2 final hidden [B,8] fp32.

Strategy (pure data parallel over 8 cores, B=131072 -> 16384/core):
  - Host: embed tokens (tiny 27x8 table gather) and pack activations into
    tile layout [T, NBLK, 128, NJ] where partition p = 8*g + h packs 16
    independent sequence groups of the 8-wide hidden dim, and the free dim
    j indexes NJ sequences per group. One [128, NJ] tile = 16*NJ sequences.
  - Device: all four gate matmuls (input-side and hidden-side, both
    layers) are block-diagonal [128,128] x [128,NJ] PE matmuls
    accumulating in PSUM (gate-pure banks R, Z', Ni, Nh). All biases fold
    into ACT per-partition bias / scalar_tensor_tensor scalars.
    z' trick: weights/biases of the z gate are negated so
    sigmoid gives z' = 1 - z and h' = h + z' * (n - h).
  - Engine split per cell: PE 6 matmuls; ACT sigmoid r, sigmoid z',
    tanh n; DVE stt u=(Nh+b)*r, v=Ni+u, d=n-h; GPSIMD e=z'*d, h'=h+e.
"""

import numpy as np

import concourse.bacc as bacc
import concourse.tile as tile
from concourse import mybir
from concourse.bass_utils import run_bass_kernel_spmd

FP32 = mybir.dt.float32
FP16 = mybir.dt.float16
AF = mybir.ActivationFunctionType
OP = mybir.AluOpType

H = 8
VOCAB = 27
N_CORES = 8
B_FULL = 131072
T_FULL = 24
G = 16          # sequence groups per tile (16 * 8 = 128 partitions)
NJ_FULL = 512   # sequences per group per tile (one PSUM bank of fp32)
NBLK_FULL = 2   # tiles per core: 2 * 16 * 512 = 16384 seqs/core


def build_program(T=T_FULL, NBLK=NBLK_FULL, NJ=NJ_FULL):
    NW = NBLK * NJ  # wide free dim: all blocks merged for non-matmul ops
    nc = bacc.Bacc(None, target_bir_lowering=False)
    x_d = nc.declare_dram_parameter("x", [T, 128, NW], FP16, isOutput=False)
    w_d = nc.declare_dram_parameter("w", [128, 12 * 128], FP16, isOutput=False)
    b_d = nc.declare_dram_parameter("b", [128, 8], FP32, isOutput=False)
    o_d = nc.declare_dram_parameter("out", [128, NW], FP16, isOutput=True)

    def halves():
        return [slice(i * NJ, (i + 1) * NJ) for i in range(NBLK)]

    with tile.TileContext(nc) as tc:
        with (
            tc.tile_pool(name="wpool", bufs=1) as wpool,
            tc.tile_pool(name="hpool", bufs=1) as hpool,
            tc.tile_pool(name="xpool", bufs=3) as xpool,
            tc.tile_pool(name="tpool", bufs=3) as tpool,
            tc.tile_pool(name="psum", bufs=1, space="PSUM") as psum,
        ):
            wt = wpool.tile([128, 12 * 128], FP16, name="wt")
            bt = wpool.tile([128, 8], FP32, name="bt")
            nc.sync.dma_start(wt[:], w_d[:])
            nc.sync.dma_start(bt[:], b_d[:])

            def W(i):
                return wt[:, 128 * i : 128 * (i + 1)]

            def Bc(i):
                return bt[:, i : i + 1]

            h = {}
            for L in (1, 2):
                for par in (0, 1):
                    h[(L, par)] = hpool.tile([128, NW], FP16, name=f"h{L}_{par}")

            def cell(L, t, xin, Hp, Hn):
                off = 0 if L == 1 else 6
                bo = 0 if L == 1 else 4
                first = t == 0
                R = psum.tile([128, NW], FP32, name="Rp")
                Z = psum.tile([128, NW], FP32, name="Zp")
                NI = psum.tile([128, NW], FP32, name="NIp")
                # matmuls at FD=NJ per PSUM half-bank; same lhsT back-to-back
                # so weight loads amortize over both halves
                for g, dst in ((0, R), (1, Z), (2, NI)):
                    for s in halves():
                        nc.tensor.matmul(
                            dst[:, s], W(off + g), xin[:, s],
                            start=True, stop=first or g == 2,
                        )
                if not first:
                    NH = psum.tile([128, NW], FP32, name="NHp")
                    for g, dst, st in ((3, R, False), (4, Z, False), (5, NH, True)):
                        for s in halves():
                            nc.tensor.matmul(
                                dst[:, s], W(off + g), Hp[:, s],
                                start=st, stop=True,
                            )
                r = tpool.tile([128, NW], FP16, name="r")
                z = tpool.tile([128, NW], FP16, name="z")
                nc.scalar.activation(r[:], R[:], AF.Sigmoid, bias=Bc(bo + 0))
                nc.scalar.activation(z[:], Z[:], AF.Sigmoid, bias=Bc(bo + 1))
                u = tpool.tile([128, NW], FP32, name="u")
                if first:
                    nc.vector.tensor_scalar_mul(u[:], r[:], Bc(bo + 2))
                else:
                    nc.vector.scalar_tensor_tensor(
                        u[:], NH[:], Bc(bo + 2), r[:], op0=OP.add, op1=OP.mult
                    )
                nc.vector.tensor_tensor(NI[:], NI[:], u[:], op=OP.add)
                n = tpool.tile([128, NW], FP16, name="n")
                nc.scalar.activation(n[:], NI[:], AF.Tanh, bias=Bc(bo + 3))
                if first:
                    nc.gpsimd.tensor_tensor(Hn[:], z[:], n[:], op=OP.mult)
                else:
                    d = tpool.tile([128, NW], FP16, name="d")
                    nc.vector.tensor_sub(d[:], n[:], Hp[:])
                    e = tpool.tile([128, NW], FP16, name="e")
                    nc.vector.tensor_tensor(e[:], z[:], d[:], op=OP.mult)
                    nc.gpsimd.tensor_tensor(Hn[:], Hp[:], e[:], op=OP.add)

            # Software-pipelined wavefront: layer 1 runs one timestep ahead of
            # layer 2 — cell(1, t+1) and cell(2, t) are independent.
            xt = xpool.tile([128, NW], FP16, name="xt")
            nc.sync.dma_start(xt[:], x_d[0])
            cell(1, 0, xt, h[(1, 0)], h[(1, 1)])
            for t in range(T):
                if t + 1 < T:
                    xt = xpool.tile([128, NW], FP16, name="xt")
                    nc.sync.dma_start(xt[:], x_d[t + 1])
                    cell(1, t + 1, xt, h[(1, (t + 1) % 2)], h[(1, t % 2)])
                cell(2, t, h[(1, (t + 1) % 2)], h[(2, t % 2)], h[(2, (t + 1) % 2)])

            nc.sync.dma_start(o_d[:], h[(2, T % 2)][:])

    return nc


def _block_diag_lhsT(Wg, negate=False):
    # Wg: [8, 8] gate block (rows = output h, cols = input h).
    # lhsT[k, m] = Wg[m, k]; block-diag over 16 groups.
    A = Wg.T.astype(np.float32)
    if negate:
        A = -A
    return np.kron(np.eye(G, dtype=np.float32), A)


def pack_weights(w_ih1, w_hh1, b_ih1, b_hh1, w_ih2, w_hh2, b_ih2, b_hh2):
    mats = []
    for Wfull in (w_ih1, w_hh1, w_ih2, w_hh2):
        Wfull = np.asarray(Wfull, dtype=np.float32)
        for gate in range(3):
            blkm = Wfull[8 * gate : 8 * gate + 8, :]
            mats.append(_block_diag_lhsT(blkm, negate=(gate == 1)))
    wblob = np.ascontiguousarray(
        np.concatenate(mats, axis=1).astype(np.float16)
    )  # [128, 1536]

    b_ih1 = np.asarray(b_ih1, np.float32)
    b_hh1 = np.asarray(b_hh1, np.float32)
    b_ih2 = np.asarray(b_ih2, np.float32)
    b_hh2 = np.asarray(b_hh2, np.float32)

    def t16(v):
        return np.tile(v.astype(np.float32), G)

    cols = [
        t16(b_ih1[0:8] + b_hh1[0:8]),        # sigmoid bias r, L1
        t16(-(b_ih1[8:16] + b_hh1[8:16])),   # sigmoid bias z' (negated), L1
        t16(b_hh1[16:24]),                   # stt scalar (b_hh n), L1
        t16(b_ih1[16:24]),                   # tanh bias (b_ih n), L1
        t16(b_ih2[0:8] + b_hh2[0:8]),
        t16(-(b_ih2[8:16] + b_hh2[8:16])),
        t16(b_hh2[16:24]),
        t16(b_ih2[16:24]),
    ]
    bblob = np.ascontiguousarray(np.stack(cols, axis=1))  # [128, 8]
    return wblob, bblob


def pack_x(tokens, emb, n_cores=N_CORES, T=T_FULL, NBLK=NBLK_FULL, NJ=NJ_FULL):
    # tokens [B, T] int, emb [27, 8]; returns [n_cores, T, 128, NBLK*NJ] fp16
    tokens = np.asarray(tokens).astype(np.int64)
    emb_eff = np.asarray(emb, dtype=np.float32).copy()
    emb_eff[0] = 0.0
    x_full = emb_eff[tokens]  # [B, T, 8]
    B = tokens.shape[0]
    assert B == n_cores * NBLK * G * NJ and tokens.shape[1] == T
    xp = x_full.reshape(n_cores, NBLK, G, NJ, T, H)
    xp = xp.transpose(0, 4, 2, 5, 1, 3)  # [c, t, g, h, blk, j]
    return np.ascontiguousarray(
        xp.reshape(n_cores, T, 128, NBLK * NJ).astype(np.float16)
    )


def unpack_out(outs, n_cores=N_CORES, NBLK=NBLK_FULL, NJ=NJ_FULL):
    # outs: list of [128, NBLK*NJ] per core -> [B, 8]
    o = np.stack([np.asarray(x) for x in outs]).astype(np.float32)
    o = o.reshape(n_cores, G, H, NBLK, NJ).transpose(0, 3, 1, 4, 2)
    return np.ascontiguousarray(o.reshape(n_cores * NBLK * G * NJ, H))


def run(inputs, trace=False, **spmd_kwargs):
    xp = pack_x(inputs["inputs"], inputs["emb"])
    wblob, bblob = pack_weights(
        inputs["w_ih1"], inputs["w_hh1"], inputs["b_ih1"], inputs["b_hh1"],
        inputs["w_ih2"], inputs["w_hh2"], inputs["b_ih2"], inputs["b_hh2"],
    )
    nc = build_program()
    nc.finalize()
    in_maps = [
        {"x": np.ascontiguousarray(xp[c]), "w": wblob, "b": bblob}
        for c in range(N_CORES)
    ]
    res = run_bass_kernel_spmd(
        nc, in_maps, list(range(N_CORES)), trace=trace, **spmd_kwargs
    )
    out = unpack_out([res.results[c]["out"] for c in range(N_CORES)])
    return out, res


def kernel(**inputs) -> np.ndarray:
    out, _ = run(inputs)
    return out


# revision 21
# speedup vs baseline: 1.1271x; 1.1271x over previous
"""Trainium2 Bass kernel for nn_PinyinGRUEmbeddings.

Math: x = emb_eff[tokens] ([B,T,8], emb row 0 zeroed), two stacked GRU
layers (torch gate order r,z,n), output = layer-2 final hidden [B,8] fp32.

Strategy (pure data parallel over 8 cores, B=131072 -> 16384/core):
  - Host: embed tokens (tiny 27x8 table gather) and pack activations into
    tile layout [T, NBLK, 128, NJ] where partition p = 8*g + h packs 16
    independent sequence groups of the 8-wide hidden dim, and the free dim
    j indexes NJ sequences per group. One [128, NJ] tile = 16*NJ sequences.
  - Device: all four gate matmuls (input-side and hidden-side, both
    layers) are block-diagonal [128,128] x [128,NJ] PE matmuls
    accumulating in PSUM (gate-pure banks R, Z', Ni, Nh). All biases fold
    into ACT per-partition bias / scalar_tensor_tensor scalars.
    z' trick: weights/biases of the z gate are negated so
    sigmoid gives z' = 1 - z and h' = h + z' * (n - h).
  - Engine split per cell: PE 6 matmuls; ACT sigmoid r, sigmoid z',
    tanh n; DVE stt u=(Nh+b)*r, v=Ni+u, d=n-h; GPSIMD e=z'*d, h'=h+e.
"""

import numpy as np

import concourse.bacc as bacc
import concourse.tile as tile
from concourse import mybir
from concourse.bass_utils import run_bass_kernel_spmd

FP32 = mybir.dt.float32
FP16 = mybir.dt.float16
AF = mybir.ActivationFunctionType
OP = mybir.AluOpType

H = 8
VOCAB = 27
N_CORES = 8
B_FULL = 131072
T_FULL = 24
G = 16          # sequence groups per tile (16 * 8 = 128 partitions)
NJ_FULL = 512   # sequences per group per tile (one PSUM bank of fp32)
NBLK_FULL = 2   # tiles per core: 2 * 16 * 512 = 16384 seqs/core


def build_program(T=T_FULL, NBLK=NBLK_FULL, NJ=NJ_FULL):
    NW = NBLK * NJ  # wide free dim: all blocks merged for non-matmul ops
    nc = bacc.Bacc(None, target_bir_lowering=False)
    x_d = nc.declare_dram_parameter("x", [T, 128, NW], FP16, isOutput=False)
    w_d = nc.declare_dram_parameter("w", [128, 12 * 128], FP16, isOutput=False)
    b_d = nc.declare_dram_parameter("b", [128, 8], FP32, isOutput=False)
    bh_d = nc.declare_dram_parameter("bh", [128, 8], FP16, isOutput=False)
    o_d = nc.declare_dram_parameter("out", [128, NW], FP16, isOutput=True)

    HS = [slice(i * NJ, (i + 1) * NJ) for i in range(NBLK)]

    with tile.TileContext(nc) as tc:
        with (
            tc.tile_pool(name="wpool", bufs=1) as wpool,
            tc.tile_pool(name="hpool", bufs=1) as hpool,
            tc.tile_pool(name="xpool", bufs=3) as xpool,
            tc.tile_pool(name="tpool", bufs=4) as tpool,
            tc.tile_pool(name="psum", bufs=2, space="PSUM") as psum,
        ):
            wt = wpool.tile([128, 12 * 128], FP16, name="wt")
            bt = wpool.tile([128, 8], FP32, name="bt")
            btH = wpool.tile([128, 8], FP16, name="btH")
            nc.sync.dma_start(wt[:], w_d[:])
            nc.sync.dma_start(bt[:], b_d[:])
            nc.sync.dma_start(btH[:], bh_d[:])

            def W(i):
                return wt[:, 128 * i : 128 * (i + 1)]

            def Bc(i):
                return bt[:, i : i + 1]

            def BcH(i):
                return btH[:, i : i + 1]

            h = {}
            for L in (1, 2):
                for par in (0, 1):
                    h[(L, par)] = hpool.tile([128, NW], FP16, name=f"h{L}_{par}")

            # Each cell time-shares two PSUM tile-pairs: pB hosts R then NI,
            # pA hosts NH then Z. With bufs=2, the two in-flight wavefront
            # instances (L1@t+1, L2@t) get disjoint banks, so PE never stalls
            # on the other instance's consumers. Cells are emitted as six
            # phases, interleaved across the two instances per round.
            def cell(L, t, xin, Hp, Hn):
                off = 0 if L == 1 else 6
                bo = 0 if L == 1 else 4
                first = t == 0
                st = {}

                def p1():
                    st["pB"] = psum.tile([128, NW], FP32, name="pB")
                    st["pA"] = psum.tile([128, NW], FP32, name="pA")
                    for s in HS:
                        nc.tensor.matmul(
                            st["pB"][:, s], W(off + 0), xin[:, s],
                            start=True, stop=first,
                        )
                    if not first:
                        for s in reversed(HS):
                            nc.tensor.matmul(
                                st["pB"][:, s], W(off + 3), Hp[:, s],
                                start=False, stop=True,
                            )
                        for s in reversed(HS):
                            nc.tensor.matmul(
                                st["pA"][:, s], W(off + 5), Hp[:, s],
                                start=True, stop=True,
                            )

                def p2():
                    st["r"] = tpool.tile([128, NW], FP16, name="r")
                    nc.scalar.activation(
                        st["r"][:], st["pB"][:], AF.Sigmoid, bias=Bc(bo + 0)
                    )
                    if not first:
                        st["nh"] = tpool.tile([128, NW], FP16, name="nh")
                        nc.scalar.copy(st["nh"][:], st["pA"][:])

                def p3():
                    for s in HS:
                        nc.tensor.matmul(
                            st["pB"][:, s], W(off + 2), xin[:, s],
                            start=True, stop=True,
                        )
                    for s in HS:
                        nc.tensor.matmul(
                            st["pA"][:, s], W(off + 1), xin[:, s],
                            start=True, stop=first,
                        )
                    if not first:
                        for s in reversed(HS):
                            nc.tensor.matmul(
                                st["pA"][:, s], W(off + 4), Hp[:, s],
                                start=False, stop=True,
                            )

                def p4():
                    u = tpool.tile([128, NW], FP16, name="u")
                    if first:
                        nc.vector.tensor_scalar_mul(u[:], st["r"][:], Bc(bo + 2))
                    else:
                        nc.vector.scalar_tensor_tensor(
                            u[:], st["nh"][:], BcH(bo + 2), st["r"][:],
                            op0=OP.add, op1=OP.mult,
                        )
                    nc.vector.tensor_tensor(st["pB"][:], st["pB"][:], u[:], op=OP.add)

                def p5():
                    st["z"] = tpool.tile([128, NW], FP16, name="z")
                    nc.scalar.activation(
                        st["z"][:], st["pA"][:], AF.Sigmoid, bias=Bc(bo + 1)
                    )
                    st["n"] = tpool.tile([128, NW], FP16, name="n")
                    nc.scalar.activation(
                        st["n"][:], st["pB"][:], AF.Tanh, bias=Bc(bo + 3)
                    )

                def p6():
                    z, n = st["z"], st["n"]
                    if first:
                        nc.gpsimd.tensor_tensor(
                            Hn[:, HS[0]], z[:, HS[0]], n[:, HS[0]], op=OP.mult
                        )
                        nc.vector.tensor_tensor(
                            Hn[:, HS[-1]], z[:, HS[-1]], n[:, HS[-1]], op=OP.mult
                        )
                    else:
                        d = tpool.tile([128, NW], FP16, name="d")
                        nc.vector.tensor_sub(d[:], n[:], Hp[:])
                        e = tpool.tile([128, NW], FP16, name="e")
                        nc.vector.tensor_tensor(e[:], z[:], d[:], op=OP.mult)
                        nc.gpsimd.tensor_tensor(
                            Hn[:, HS[0]], Hp[:, HS[0]], e[:, HS[0]], op=OP.add
                        )
                        nc.vector.tensor_tensor(
                            Hn[:, HS[-1]], Hp[:, HS[-1]], e[:, HS[-1]], op=OP.add
                        )

                return [p1, p2, p3, p4, p5, p6]

            xt = xpool.tile([128, NW], FP16, name="xt")
            nc.sync.dma_start(xt[:], x_d[0])
            for p in cell(1, 0, xt, h[(1, 0)], h[(1, 1)]):
                p()
            for t in range(T):
                insts = []
                if t + 1 < T:
                    xt = xpool.tile([128, NW], FP16, name="xt")
                    nc.sync.dma_start(xt[:], x_d[t + 1])
                    insts.append(cell(1, t + 1, xt, h[(1, (t + 1) % 2)], h[(1, t % 2)]))
                insts.append(
                    cell(2, t, h[(1, (t + 1) % 2)], h[(2, t % 2)], h[(2, (t + 1) % 2)])
                )
                for i in range(6):
                    for inst in insts:
                        inst[i]()

            nc.sync.dma_start(o_d[:], h[(2, T % 2)][:])

    return nc


def _block_diag_lhsT(Wg, negate=False):
    # Wg: [8, 8] gate block (rows = output h, cols = input h).
    # lhsT[k, m] = Wg[m, k]; block-diag over 16 groups.
    A = Wg.T.astype(np.float32)
    if negate:
        A = -A
    return np.kron(np.eye(G, dtype=np.float32), A)


def pack_weights(w_ih1, w_hh1, b_ih1, b_hh1, w_ih2, w_hh2, b_ih2, b_hh2):
    mats = []
    for Wfull in (w_ih1, w_hh1, w_ih2, w_hh2):
        Wfull = np.asarray(Wfull, dtype=np.float32)
        for gate in range(3):
            blkm = Wfull[8 * gate : 8 * gate + 8, :]
            mats.append(_block_diag_lhsT(blkm, negate=(gate == 1)))
    wblob = np.ascontiguousarray(
        np.concatenate(mats, axis=1).astype(np.float16)
    )  # [128, 1536]

    b_ih1 = np.asarray(b_ih1, np.float32)
    b_hh1 = np.asarray(b_hh1, np.float32)
    b_ih2 = np.asarray(b_ih2, np.float32)
    b_hh2 = np.asarray(b_hh2, np.float32)

    def t16(v):
        return np.tile(v.astype(np.float32), G)

    cols = [
        t16(b_ih1[0:8] + b_hh1[0:8]),        # sigmoid bias r, L1
        t16(-(b_ih1[8:16] + b_hh1[8:16])),   # sigmoid bias z' (negated), L1
        t16(b_hh1[16:24]),                   # stt scalar (b_hh n), L1
        t16(b_ih1[16:24]),                   # tanh bias (b_ih n), L1
        t16(b_ih2[0:8] + b_hh2[0:8]),
        t16(-(b_ih2[8:16] + b_hh2[8:16])),
        t16(b_hh2[16:24]),
        t16(b_ih2[16:24]),
    ]
    bblob = np.ascontiguousarray(np.stack(cols, axis=1))  # [128, 8]
    return wblob, bblob


def pack_x(tokens, emb, n_cores=N_CORES, T=T_FULL, NBLK=NBLK_FULL, NJ=NJ_FULL):
    # tokens [B, T] int, emb [27, 8]; returns [n_cores, T, 128, NBLK*NJ] fp16
    tokens = np.asarray(tokens).astype(np.int64)
    emb_eff = np.asarray(emb, dtype=np.float32).copy()
    emb_eff[0] = 0.0
    x_full = emb_eff[tokens]  # [B, T, 8]
    B = tokens.shape[0]
    assert B == n_cores * NBLK * G * NJ and tokens.shape[1] == T
    xp = x_full.reshape(n_cores, NBLK, G, NJ, T, H)
    xp = xp.transpose(0, 4, 2, 5, 1, 3)  # [c, t, g, h, blk, j]
    return np.ascontiguousarray(
        xp.reshape(n_cores, T, 128, NBLK * NJ).astype(np.float16)
    )


def unpack_out(outs, n_cores=N_CORES, NBLK=NBLK_FULL, NJ=NJ_FULL):
    # outs: list of [128, NBLK*NJ] per core -> [B, 8]
    o = np.stack([np.asarray(x) for x in outs]).astype(np.float32)
    o = o.reshape(n_cores, G, H, NBLK, NJ).transpose(0, 3, 1, 4, 2)
    return np.ascontiguousarray(o.reshape(n_cores * NBLK * G * NJ, H))


def run(inputs, trace=False, **spmd_kwargs):
    xp = pack_x(inputs["inputs"], inputs["emb"])
    wblob, bblob = pack_weights(
        inputs["w_ih1"], inputs["w_hh1"], inputs["b_ih1"], inputs["b_hh1"],
        inputs["w_ih2"], inputs["w_hh2"], inputs["b_ih2"], inputs["b_hh2"],
    )
    nc = build_program()
    nc.finalize()
    bhblob = bblob.astype(np.float16)
    in_maps = [
        {"x": np.ascontiguousarray(xp[c]), "w": wblob, "b": bblob, "bh": bhblob}
        for c in range(N_CORES)
    ]
    res = run_bass_kernel_spmd(
        nc, in_maps, list(range(N_CORES)), trace=trace, **spmd_kwargs
    )
    out = unpack_out([res.results[c]["out"] for c in range(N_CORES)])
    return out, res


def kernel(**inputs) -> np.ndarray:
    out, _ = run(inputs)
    return out
